# revision 1
# baseline (speedup 1.0000x reference)
"""Trainium2 Bass kernel for nn_CAttention (channel attention).

Reference computation (per batch b):
    k      = einsum('cit,i->ct', x[b], alpha)          # [C, T]
    scores = k @ W @ k.T                               # [C, C]
    att    = softmax(scores, axis=-1)
    out[b] = att @ x[b].reshape(C, N*T)                # [C, N*T]

Shapes (hardcoded): x [64, 256, 307, 12] f32, W [12, 12], alpha [307].
Sharding: data-parallel over batch B across 8 cores (8 batches/core);
W and alpha replicated.
"""

from contextlib import ExitStack

import numpy as np

import concourse.bass as bass
import concourse.tile as tile
from concourse import bacc, mybir
from concourse.bass import ts
from concourse.bass_utils import run_bass_kernel_spmd
from concourse.masks import make_identity

B, C, N, T = 64, 256, 307, 12
NCORES = 8
B_LOC = B // NCORES          # 8 batches per core
F = N * T                    # 3684 flattened free dim
P = 128                      # partitions
CC = C // P                  # 2 c-chunks
FT = 512                     # f-tile size for the big matmul
F_TILES = [(f0, min(FT, F - f0)) for f0 in range(0, F, FT)]

_DT = mybir.dt.float32


def _emit_core_kernel(tc, x_ap, w_ap, alpha_ap, out_ap):
    """Emit the per-core program. x_ap/out_ap: [B_LOC, C, N, T] DRAM."""
    nc = tc.nc
    ctx = ExitStack()

    x_flat = x_ap.rearrange("b c i t -> b c (i t)")      # [B_LOC, C, F]
    out_flat = out_ap.rearrange("b c i t -> b c (i t)")  # [B_LOC, C, F]

    consts = ctx.enter_context(tc.tile_pool(name="consts", bufs=1))
    xpool = ctx.enter_context(tc.tile_pool(name="x", bufs=3))
    xapool = ctx.enter_context(tc.tile_pool(name="xa", bufs=2))
    kpool = ctx.enter_context(tc.tile_pool(name="k", bufs=2))
    ktpool = ctx.enter_context(tc.tile_pool(name="kt", bufs=2))
    attpool = ctx.enter_context(tc.tile_pool(name="att", bufs=2))
    outpool = ctx.enter_context(tc.tile_pool(name="out", bufs=4))
    psmall = ctx.enter_context(tc.tile_pool(name="psmall", bufs=3, space="PSUM"))
    pscore = ctx.enter_context(tc.tile_pool(name="pscore", bufs=2, space="PSUM"))
    pbig = ctx.enter_context(tc.tile_pool(name="pbig", bufs=3, space="PSUM"))

    # Constants: identity for PE transpose, alpha broadcast to all partitions, W.
    ident = consts.tile([P, P], _DT)
    make_identity(nc, ident)
    alpha_row = consts.tile([P, N], _DT)
    nc.gpsimd.dma_start(out=alpha_row, in_=alpha_ap[None, :].to_broadcast([P, N]))
    w_sb = consts.tile([T, T], _DT)
    nc.gpsimd.dma_start(out=w_sb, in_=w_ap)

    def phase1(b):
        """Load x[b]; compute att (unnormalized) and 1/rowsum."""
        x_t = xpool.tile([P, CC, F], _DT, tag="x")
        for cc in range(CC):
            nc.sync.dma_start(out=x_t[:, cc, :], in_=x_flat[b, ts(cc, P), :])

        # k[c, t] = sum_i alpha[i] * x[c, i, t]
        k_c = kpool.tile([P, CC, T], _DT, tag="k")
        kt_sb = ktpool.tile([T, C], _DT, tag="kt")
        for cc in range(CC):
            xa = xapool.tile([P, N, T], _DT, tag="xa")
            nc.vector.tensor_mul(
                xa,
                x_t[:, cc, :].rearrange("p (i t) -> p i t", t=T),
                alpha_row[:, :, None].to_broadcast([P, N, T]),
            )
            nc.vector.reduce_sum(
                out=k_c[:, cc, :],
                in_=xa.rearrange("p i t -> p t i"),
                axis=mybir.AxisListType.X,
            )
            # kT[t, c-chunk] via PE transpose
            ps_kt = psmall.tile([P, 2 * P], _DT, tag="small")
            nc.tensor.transpose(ps_kt[:T, :P], k_c[:, cc, :], ident)
            nc.scalar.copy(out=kt_sb[:, ts(cc, P)], in_=ps_kt[:T, :P])

        # kWT[s, c] = sum_t W[t, s] kT[t, c]
        ps_kwt = psmall.tile([P, 2 * P], _DT, tag="small")
        nc.tensor.matmul(ps_kwt[:T, :C], lhsT=w_sb, rhs=kt_sb, start=True, stop=True)
        kwt_sb = ktpool.tile([T, C], _DT, tag="kwt")
        nc.scalar.copy(out=kwt_sb, in_=ps_kwt[:T, :C])

        # scores[c, d] = sum_s kWT[s, c] kT[s, d]; softmax over d
        att_sb = attpool.tile([P, CC, C], _DT, tag="att")
        stats = kpool.tile([P, CC, 3], _DT, tag="stats")  # [negmax, sumexp, rinv]
        for cc in range(CC):
            ps_sc = pscore.tile([P, C], _DT, tag="score")
            nc.tensor.matmul(
                ps_sc, lhsT=kwt_sb[:, ts(cc, P)], rhs=kt_sb, start=True, stop=True
            )
            nc.vector.tensor_reduce(
                out=stats[:, cc, 0:1],
                in_=ps_sc,
                axis=mybir.AxisListType.X,
                op=mybir.AluOpType.max,
                negate=True,
            )
            nc.scalar.activation(
                out=att_sb[:, cc, :],
                in_=ps_sc,
                func=mybir.ActivationFunctionType.Exp,
                bias=stats[:, cc, 0:1],
                scale=1.0,
                accum_out=stats[:, cc, 1:2],
            )
            nc.vector.reciprocal(out=stats[:, cc, 2:3], in_=stats[:, cc, 1:2])
        return {"x_t": x_t, "att_sb": att_sb, "stats": stats}

    def phase2(b, st):
        """Transpose att, big matmul out = attT.T @ x, scale rows, store."""
        x_t, att_sb, stats = st["x_t"], st["att_sb"], st["stats"]

        att_t = attpool.tile([P, CC, C], _DT, tag="attT")
        for dc in range(CC):
            for cc in range(CC):
                ps_at = psmall.tile([P, 2 * P], _DT, tag="small")
                nc.tensor.transpose(
                    ps_at[:, :P], att_sb[:, cc, ts(dc, P)], ident
                )
                nc.scalar.copy(out=att_t[:, dc, ts(cc, P)], in_=ps_at[:, :P])

        for cc in range(CC):
            for f0, fsz in F_TILES:
                ps_o = pbig.tile([P, FT], _DT, tag="big")
                for dc in range(CC):
                    nc.tensor.matmul(
                        ps_o[:, :fsz],
                        lhsT=att_t[:, dc, ts(cc, P)],
                        rhs=x_t[:, dc, f0 : f0 + fsz],
                        start=(dc == 0),
                        stop=(dc == CC - 1),
                    )
                o_sb = outpool.tile([P, FT], _DT, tag="o")
                nc.scalar.mul(
                    out=o_sb[:, :fsz], in_=ps_o[:, :fsz], mul=stats[:, cc, 2:3]
                )
                nc.sync.dma_start(
                    out=out_flat[b, ts(cc, P), f0 : f0 + fsz], in_=o_sb[:, :fsz]
                )

    # Software pipeline: phase1(b+1) is emitted before phase2(b) so the PE's
    # small matmuls for b+1 fill the softmax wait bubble of b.
    prev = None
    for b in range(B_LOC):
        cur = phase1(b)
        if prev is not None:
            phase2(b - 1, prev)
        prev = cur
    phase2(B_LOC - 1, prev)
    ctx.close()


_CACHED_NC = None


def _build():
    global _CACHED_NC
    if _CACHED_NC is not None:
        return _CACHED_NC
    nc = bacc.Bacc("TRN2", target_bir_lowering=False, debug=False, num_devices=NCORES)
    x_d = nc.dram_tensor("x", [B_LOC, C, N, T], _DT, kind="ExternalInput").ap()
    w_d = nc.dram_tensor("W", [T, T], _DT, kind="ExternalInput").ap()
    a_d = nc.dram_tensor("alpha", [N], _DT, kind="ExternalInput").ap()
    o_d = nc.dram_tensor("out", [B_LOC, C, N, T], _DT, kind="ExternalOutput").ap()
    with tile.TileContext(nc) as tc:
        _emit_core_kernel(tc, x_d, w_d, a_d, o_d)
    nc.compile()
    _CACHED_NC = nc
    return nc


def run(x, W, alpha, trace=False, **spmd_kwargs):
    """Run on 8 cores; returns (full output [B,C,N,T], BassKernelResults)."""
    x = np.ascontiguousarray(np.asarray(x, dtype=np.float32))
    W = np.ascontiguousarray(np.asarray(W, dtype=np.float32))
    alpha = np.ascontiguousarray(np.asarray(alpha, dtype=np.float32))
    assert x.shape == (B, C, N, T) and W.shape == (T, T) and alpha.shape == (N,)

    nc = _build()
    in_maps = [
        {"x": x[i * B_LOC : (i + 1) * B_LOC], "W": W, "alpha": alpha}
        for i in range(NCORES)
    ]
    res = run_bass_kernel_spmd(
        nc, in_maps, core_ids=list(range(NCORES)), trace=trace, **spmd_kwargs
    )
    out = np.concatenate([r["out"] for r in res.results], axis=0)
    return out, res


def kernel(x, W, alpha):
    out, _ = run(x, W, alpha)
    return out


# revision 9
# speedup vs baseline: 1.1294x; 1.1294x over previous
"""Trainium2 Bass kernel for nn_CAttention (channel attention).

Reference computation (per batch b):
    k      = einsum('cit,i->ct', x[b], alpha)          # [C, T]
    scores = k @ W @ k.T                               # [C, C]
    att    = softmax(scores, axis=-1)
    out[b] = att @ x[b].reshape(C, N*T)                # [C, N*T]

Shapes (hardcoded): x [64, 256, 307, 12] f32, W [12, 12], alpha [307].
Sharding: data-parallel over batch B across 8 cores (8 batches/core);
W and alpha replicated.
"""

from contextlib import ExitStack

import numpy as np

import concourse.bass as bass
import concourse.tile as tile
from concourse import bacc, mybir
from concourse.bass import ts
from concourse.bass_utils import run_bass_kernel_spmd
from concourse.masks import make_identity

B, C, N, T = 64, 256, 307, 12
NCORES = 8
B_LOC = B // NCORES          # 8 batches per core
F = N * T                    # 3684 flattened free dim
P = 128                      # partitions
CC = C // P                  # 2 c-chunks
FT = 512                     # f-tile size for the big matmul
F_TILES = [(f0, min(FT, F - f0)) for f0 in range(0, F, FT)]

_DT = mybir.dt.float32
# float32r (fp32 with the low 12 mantissa bits dropped at the PE input)
# streams through the PE in a single pass (1 cycle/row for free dims
# >= 256) instead of the two half-speed passes plain float32 lowers to.
# Only the big output matmul uses it; the tiny score matmuls stay float32
# since softmax amplifies logit error.  x is DMA'd straight into a
# float32r-typed tile — the bits stay full fp32, so the k-path reads the
# same tile bitcast back to float32 at full precision while the PE
# truncates only when streaming the big matmul.
_R = mybir.dt.float32r


def _emit_core_kernel(tc, x_ap, w_ap, alpha_ap, out_ap):
    """Emit the per-core program. x_ap/out_ap: [B_LOC, C, N, T] DRAM."""
    nc = tc.nc
    ctx = ExitStack()

    x_flat = x_ap.rearrange("b c i t -> b c (i t)")      # [B_LOC, C, F]
    out_flat = out_ap.rearrange("b c i t -> b c (i t)")  # [B_LOC, C, F]

    consts = ctx.enter_context(tc.tile_pool(name="consts", bufs=1))
    xpool = ctx.enter_context(tc.tile_pool(name="x", bufs=3))
    xapool = ctx.enter_context(tc.tile_pool(name="xa", bufs=2))
    kpool = ctx.enter_context(tc.tile_pool(name="k", bufs=2))
    ktpool = ctx.enter_context(tc.tile_pool(name="kt", bufs=2))
    attpool = ctx.enter_context(tc.tile_pool(name="att", bufs=2))
    outpool = ctx.enter_context(tc.tile_pool(name="out", bufs=4))
    psmall = ctx.enter_context(tc.tile_pool(name="psmall", bufs=3, space="PSUM"))
    pscore = ctx.enter_context(tc.tile_pool(name="pscore", bufs=2, space="PSUM"))
    pbig = ctx.enter_context(tc.tile_pool(name="pbig", bufs=3, space="PSUM"))

    # Constants: identity for PE transpose, alpha broadcast to all partitions, W.
    ident = consts.tile([P, P], _DT)
    make_identity(nc, ident)
    alpha_row = consts.tile([P, N], _DT)
    nc.gpsimd.dma_start(out=alpha_row, in_=alpha_ap[None, :].to_broadcast([P, N]))
    w_sb = consts.tile([T, T], _DT)
    nc.gpsimd.dma_start(out=w_sb, in_=w_ap)

    def phase1(b):
        """Load x[b]; compute att (unnormalized) and 1/rowsum."""
        x_t = xpool.tile([P, CC, F], _R, tag="x")
        for cc in range(CC):
            nc.sync.dma_start(out=x_t[:, cc, :], in_=x_flat[b, ts(cc, P), :].bitcast(_R))

        # k[c, t] = sum_i alpha[i] * x[c, i, t]
        # The alpha-multiply runs on GpSimd (Pool) writing a t-major scratch
        # so the DVE reduction reads unit-stride; this splits the two big
        # elementwise passes across two engines.
        k_c = kpool.tile([P, CC, T], _DT, tag="k")
        kt_sb = ktpool.tile([T, C], _DT, tag="kt")
        for cc in range(CC):
            xa = xapool.tile([P, T, N], _DT, tag="xa")
            nc.gpsimd.tensor_mul(
                xa.rearrange("p t i -> p i t"),
                x_t[:, cc, :].bitcast(_DT).rearrange("p (i t) -> p i t", t=T),
                alpha_row[:, :, None].to_broadcast([P, N, T]),
            )
            nc.vector.reduce_sum(
                out=k_c[:, cc, :],
                in_=xa,
                axis=mybir.AxisListType.X,
            )
            # kT[t, c-chunk] via PE transpose
            ps_kt = psmall.tile([P, 2 * P], _DT, tag="small")
            nc.tensor.transpose(ps_kt[:T, :P], k_c[:, cc, :], ident)
            nc.scalar.copy(out=kt_sb[:, ts(cc, P)], in_=ps_kt[:T, :P])

        # kWT[s, c] = sum_t W[t, s] kT[t, c]
        ps_kwt = psmall.tile([P, 2 * P], _DT, tag="small")
        nc.tensor.matmul(ps_kwt[:T, :C], lhsT=w_sb, rhs=kt_sb, start=True, stop=True)
        kwt_sb = ktpool.tile([T, C], _DT, tag="kwt")
        nc.scalar.copy(out=kwt_sb, in_=ps_kwt[:T, :C])

        # scores[c, d] = sum_s kWT[s, c] kT[s, d]; softmax over d
        att_sb = attpool.tile([P, CC, C], _DT, tag="att")
        stats = kpool.tile([P, CC, 3], _DT, tag="stats")  # [negmax, sumexp, rinv]
        for cc in range(CC):
            ps_sc = pscore.tile([P, C], _DT, tag="score")
            nc.tensor.matmul(
                ps_sc, lhsT=kwt_sb[:, ts(cc, P)], rhs=kt_sb, start=True, stop=True
            )
            nc.vector.tensor_reduce(
                out=stats[:, cc, 0:1],
                in_=ps_sc,
                axis=mybir.AxisListType.X,
                op=mybir.AluOpType.max,
                negate=True,
            )
            nc.scalar.activation(
                out=att_sb[:, cc, :],
                in_=ps_sc,
                func=mybir.ActivationFunctionType.Exp,
                bias=stats[:, cc, 0:1],
                scale=1.0,
                accum_out=stats[:, cc, 1:2],
            )
            nc.vector.reciprocal(out=stats[:, cc, 2:3], in_=stats[:, cc, 1:2])
        return {"x_t": x_t, "att_sb": att_sb, "stats": stats}

    def phase2(b, st):
        """Transpose att, big matmul out = attT.T @ x, scale rows, store."""
        x_t, att_sb, stats = st["x_t"], st["att_sb"], st["stats"]

        att_t = attpool.tile([P, CC, C], _R, tag="attT")
        for dc in range(CC):
            for cc in range(CC):
                ps_at = psmall.tile([P, 2 * P], _DT, tag="small")
                nc.tensor.transpose(
                    ps_at[:, :P], att_sb[:, cc, ts(dc, P)], ident
                )
                nc.scalar.copy(out=att_t[:, dc, ts(cc, P)], in_=ps_at[:, :P])

        for cc in range(CC):
            for fi, (f0, fsz) in enumerate(F_TILES):
                ps_o = pbig.tile([P, FT], _DT, tag="big")
                for dc in range(CC):
                    nc.tensor.matmul(
                        ps_o[:, :fsz],
                        lhsT=att_t[:, dc, ts(cc, P)],
                        rhs=x_t[:, dc, f0 : f0 + fsz],
                        start=(dc == 0),
                        stop=(dc == CC - 1),
                    )
                o_sb = outpool.tile([P, FT], _DT, tag="o")
                # Alternate the PSUM->SBUF copy (with the 1/rowsum scale
                # fused) between ACT and DVE to balance engine load.
                if (cc * len(F_TILES) + fi) % 2 == 0:
                    nc.scalar.mul(
                        out=o_sb[:, :fsz], in_=ps_o[:, :fsz], mul=stats[:, cc, 2:3]
                    )
                else:
                    nc.vector.tensor_scalar_mul(
                        out=o_sb[:, :fsz], in0=ps_o[:, :fsz], scalar1=stats[:, cc, 2:3]
                    )
                nc.sync.dma_start(
                    out=out_flat[b, ts(cc, P), f0 : f0 + fsz], in_=o_sb[:, :fsz]
                )

    # Software pipeline: phase1(b+1) is emitted before phase2(b) so the PE's
    # small matmuls for b+1 fill the softmax wait bubble of b.
    prev = None
    for b in range(B_LOC):
        cur = phase1(b)
        if prev is not None:
            phase2(b - 1, prev)
        prev = cur
    phase2(B_LOC - 1, prev)
    ctx.close()


_CACHED_NC = None


def _build():
    global _CACHED_NC
    if _CACHED_NC is not None:
        return _CACHED_NC
    nc = bacc.Bacc("TRN2", target_bir_lowering=False, debug=False, num_devices=NCORES)
    x_d = nc.dram_tensor("x", [B_LOC, C, N, T], _DT, kind="ExternalInput").ap()
    w_d = nc.dram_tensor("W", [T, T], _DT, kind="ExternalInput").ap()
    a_d = nc.dram_tensor("alpha", [N], _DT, kind="ExternalInput").ap()
    o_d = nc.dram_tensor("out", [B_LOC, C, N, T], _DT, kind="ExternalOutput").ap()
    with tile.TileContext(nc) as tc:
        _emit_core_kernel(tc, x_d, w_d, a_d, o_d)
    nc.compile()
    _CACHED_NC = nc
    return nc


def run(x, W, alpha, trace=False, **spmd_kwargs):
    """Run on 8 cores; returns (full output [B,C,N,T], BassKernelResults)."""
    x = np.ascontiguousarray(np.asarray(x, dtype=np.float32))
    W = np.ascontiguousarray(np.asarray(W, dtype=np.float32))
    alpha = np.ascontiguousarray(np.asarray(alpha, dtype=np.float32))
    assert x.shape == (B, C, N, T) and W.shape == (T, T) and alpha.shape == (N,)

    nc = _build()
    in_maps = [
        {"x": x[i * B_LOC : (i + 1) * B_LOC], "W": W, "alpha": alpha}
        for i in range(NCORES)
    ]
    res = run_bass_kernel_spmd(
        nc, in_maps, core_ids=list(range(NCORES)), trace=trace, **spmd_kwargs
    )
    out = np.concatenate([r["out"] for r in res.results], axis=0)
    return out, res


def kernel(x, W, alpha):
    out, _ = run(x, W, alpha)
    return out
